# revision 47
# baseline (speedup 1.0000x reference)
"""Trainium2 Bass kernel for batched dot-product attention.

Problem: query/keys/values [4, 4096, 64] fp32 -> softmax(Q K^T / 8) V.

Sharding: 8 cores, data parallel. Core c handles batch c//2, query rows
(c%2)*2048 : (c%2+1)*2048, and needs full K/V of its batch. Each core runs
an identical program (SPMD) on its own shard.

Shipping variant (v7e, emit_attention_v6 + mm2lag/PAT10): fp16 matmuls
(4x the fp32 PE rate), ScalarE exp is the bottleneck engine and is kept
~100% busy:
  - K^T/Q^T built by fp16 DMA-xbar block transposes in a packed layout
    (even k-chunks on SBUF partitions 0-63, odd on 64-127); Q^T replicated
    to both halves so each MM1 is a 64-contraction matmul against the
    matching half. V' = [V | ones] fp16 (the ones column makes MM2 emit the
    softmax denominator as row 64 of the accumulator). Scores are ~N(0,1)
    after the 1/sqrt(64) scale (folded into the exp), so no max
    subtraction is needed and fp16 P cannot overflow.
  - Per 512-wide q-tile, the 32 k-chunks are processed in groups sized
    [4,3,4,3,4,3,4,3,2,2] ping-ponging between a 4-bank and a 3-bank PSUM
    buffer (+1 bank for the accumulator = exactly 8). ScalarE exp reads
    scores straight from PSUM (N up to 2048 per instruction), writes fp16
    P to SBUF. The even group count keeps the ping-pong alive across
    q-tile boundaries.
  - MM2 for group g is emitted one group behind (mm2lag) so the PE FIFO
    runs the next group's MM1 batch before the exp-gated MM2 -- keeps
    ScalarE fed back-to-back.
  - Epilogue per q-tile: cast out^T to fp16, one whole-tile DMA-xbar
    transpose back to [q, 65], reciprocal * scale on DVE, one merged
    out-DMA. The last q-tile (whose tail is exposed) uses a shorter
    PE-transpose chain instead, borrowing the just-freed B score bank.
Measured (test.py, For_i slope): ~66 us/iter vs 229.7 us fp32 baseline.
"""

import math

import numpy as np

import concourse.bass as bass
import concourse.tile as tile
from concourse import bacc, mybir
from concourse.masks import make_identity

FP32 = mybir.dt.float32

# Production shard dims
B, LQ_FULL, LK, D = 4, 4096, 4096, 64
N_CORES = 8
LQ = LQ_FULL * B // N_CORES  # 2048 query rows per core


def emit_attention(tc, q, k, v, o, LQ, LK, D, QTW, GRP=2, setup_only=False):
    """Emit the per-core attention program into TileContext tc.

    q: [LQ, D], k: [LK, D], v: [LK, D] DRAM inputs; o: [LQ, D] DRAM output.
    QTW: q-tile width (free dim of the scores matmul), <= 512 for fp32.
    GRP: k-chunks per exp group (2 -> 4+2+2 PSUM banks, 3 -> 6+1+1).
    """
    nc = tc.nc
    NKC = LK // 128   # key chunks
    NQC = LQ // 128   # query chunks (for transposes)
    NQT = LQ // QTW   # q tiles
    DP = D + 1        # V width with the ones column
    SCALE = 1.0 / math.sqrt(D)
    Exp = mybir.ActivationFunctionType.Exp
    accbufs = 2 if GRP == 2 else 1

    from contextlib import ExitStack

    with ExitStack() as ctx:
        const = ctx.enter_context(tc.tile_pool(name="const", bufs=1))
        kq = ctx.enter_context(tc.tile_pool(name="kq", bufs=1))
        loadp = ctx.enter_context(tc.tile_pool(name="load", bufs=1))
        pp = ctx.enter_context(tc.tile_pool(name="p", bufs=4))
        outp = ctx.enter_context(tc.tile_pool(name="outs", bufs=4))
        ps_scores = ctx.enter_context(
            tc.tile_pool(name="ps_scores", bufs=2, space="PSUM")
        )
        ps_acc = ctx.enter_context(
            tc.tile_pool(name="ps_acc", bufs=accbufs, space="PSUM")
        )
        ps_tr = ctx.enter_context(
            tc.tile_pool(name="ps_tr", bufs=accbufs, space="PSUM")
        )

        ident = const.tile([128, 128], FP32)
        make_identity(nc, ident[:])

        # V' = [V | 1] chunks, [128, DP] each, packed side by side.
        # One DMA for all of V, one strided memset for the ones columns.
        vp = const.tile([128, NKC * DP], FP32)
        vp3 = vp[:].rearrange("p (c w) -> p c w", w=DP)
        nc.sync.dma_start(vp3[:, :, 0:D], v.rearrange("(c p) d -> p c d", p=128))
        nc.gpsimd.memset(vp3[:, :, D:DP], 1.0)

        # Natural-layout K/Q staged chunk-major with one DMA each.
        kn = loadp.tile([128, NKC * D], FP32, tag="kn")
        nc.sync.dma_start(
            kn[:].rearrange("p (c d) -> p c d", d=D),
            k.rearrange("(c p) d -> p c d", p=128),
        )
        qn = loadp.tile([128, NQC * D], FP32, tag="qn")
        nc.sync.dma_start(
            qn[:].rearrange("p (c d) -> p c d", d=D),
            q.rearrange("(c p) d -> p c d", p=128),
        )

        # K^T / Q^T via PE transposes of the natural-layout chunks.
        kt = kq.tile([D, LK], FP32)
        for c in range(NKC):
            tr = ps_scores.tile([D, 128], FP32, tag="scores")
            nc.tensor.transpose(tr[:], kn[:, c * D : (c + 1) * D], ident[:])
            nc.vector.tensor_copy(kt[:, c * 128 : (c + 1) * 128], tr[:])

        qt = kq.tile([D, LQ], FP32)
        for c in range(NQC):
            tr = ps_scores.tile([D, 128], FP32, tag="scores")
            nc.tensor.transpose(tr[:], qn[:, c * D : (c + 1) * D], ident[:])
            nc.vector.tensor_copy(qt[:, c * 128 : (c + 1) * 128], tr[:])

        if setup_only:
            return

        for ti in range(NQT):
            acc = ps_acc.tile([DP, QTW], FP32)
            ngrp = (NKC + GRP - 1) // GRP
            qs = qt[:, ti * QTW : (ti + 1) * QTW]
            for g in range(ngrp):
                gsz = min(GRP, NKC - g * GRP)
                s = ps_scores.tile([128, GRP * QTW], FP32, tag="scores")
                for i in range(gsz):
                    c = g * GRP + i
                    nc.tensor.matmul(
                        s[:, i * QTW : (i + 1) * QTW],
                        kt[:, c * 128 : (c + 1) * 128], qs,
                        start=True, stop=True,
                    )
                p = pp.tile([128, GRP * QTW], FP32, tag="p")
                nc.scalar.activation(
                    p[:, : gsz * QTW], s[:, : gsz * QTW], Exp, scale=SCALE
                )
                for i in range(gsz):
                    c = g * GRP + i
                    nc.tensor.matmul(
                        acc[:], vp[:, c * DP : (c + 1) * DP],
                        p[:, i * QTW : (i + 1) * QTW],
                        start=(c == 0), stop=(c == NKC - 1),
                    )

            # Epilogue: transpose out^T back to [q, DP], divide by denominator.
            accs = outp.tile([DP, QTW], FP32, tag="accs")
            nc.vector.tensor_copy(accs[:], acc[:])
            for j in range(QTW // 128):
                tr = ps_tr.tile([128, DP], FP32)
                nc.tensor.transpose(tr[:], accs[:, j * 128 : (j + 1) * 128],
                                    ident[:DP, :DP])
                rc = outp.tile([128, 1], FP32, tag="rc")
                nc.vector.reciprocal(rc[:], tr[:, D:DP])
                ot = outp.tile([128, D], FP32, tag="ot")
                nc.vector.tensor_scalar_mul(ot[:], tr[:, :D], rc[:])
                r0 = ti * QTW + j * 128
                nc.sync.dma_start(o[r0 : r0 + 128, :], ot[:])


H16 = mybir.dt.float16


def emit_attention_v2(tc, q, k, v, o, LQ, LK, D, QTW, GRP=3, setup_only=False):
    """v2: bf16 K^T/Q^T via DMA xbar transposes in a packed layout
    (even k-chunks on partitions 0-63, odd on 64-127) + row-tiled
    concurrent scores matmuls + 3-chunk exp groups.

    Requires D == 64 (pair-packing trick) and QTW == 512.
    """
    nc = tc.nc
    assert D == 64 and QTW % 128 == 0
    NKC = LK // 128   # k chunks
    NQC = LQ // 128   # q chunks
    NQT = LQ // QTW
    DP = D + 1
    GRP = 3           # chunks per exp group (3 PSUM banks)
    SCALE = 1.0 / math.sqrt(D)
    Exp = mybir.ActivationFunctionType.Exp

    from contextlib import ExitStack

    with ExitStack() as ctx:
        const = ctx.enter_context(tc.tile_pool(name="const", bufs=1))
        loadp = ctx.enter_context(tc.tile_pool(name="load", bufs=1))
        pp = ctx.enter_context(tc.tile_pool(name="p", bufs=4))
        outp = ctx.enter_context(tc.tile_pool(name="outs", bufs=4))
        ps_scores = ctx.enter_context(
            tc.tile_pool(name="ps_scores", bufs=2, space="PSUM")
        )
        ps_acc = ctx.enter_context(tc.tile_pool(name="ps_acc", bufs=1, space="PSUM"))
        ps_tr = ctx.enter_context(tc.tile_pool(name="ps_tr", bufs=1, space="PSUM"))

        ident = const.tile([128, 128], FP32)
        make_identity(nc, ident[:])

        # V' = [V | 1] chunks (fp32; value path stays fp32).
        vp = const.tile([128, NKC * DP], FP32)
        vp3 = vp[:].rearrange("p (c w) -> p c w", w=DP)
        nc.sync.dma_start(vp3[:, :, 0:D], v.rearrange("(c p) d -> p c d", p=128))
        nc.gpsimd.memset(vp3[:, :, D:DP], 1.0)

        # K: load natural chunk-major, cast to bf16, xbar-transpose pair
        # blocks. Transposing kb[:, j*128:(j+1)*128] (= chunks 2j | 2j+1 side
        # by side) yields K^T of chunk 2j on partitions 0-63 and of chunk
        # 2j+1 on partitions 64-127 -- the packed row-tiling layout.
        kn = loadp.tile([128, NKC * D], FP32, tag="kn")
        nc.sync.dma_start(
            kn[:].rearrange("p (c d) -> p c d", d=D),
            k.rearrange("(c p) d -> p c d", p=128),
        )
        kb = loadp.tile([128, NKC * D], H16, tag="kb")
        nc.vector.tensor_copy(kb[:], kn[:])
        kt2 = const.tile([128, NKC * D], H16)
        for j in range(NKC // 2):
            nc.sync.dma_start(
                kt2[:, j * 128 : (j + 1) * 128],
                kb[:, j * 128 : (j + 1) * 128],
                transpose=True,
            )

        # Q: same transpose trick into alternating halves, then rearrange
        # into qt2r with Q^T replicated on both partition halves.
        qn = loadp.tile([128, NQC * D], FP32, tag="qn")
        nc.sync.dma_start(
            qn[:].rearrange("p (c d) -> p c d", d=D),
            q.rearrange("(c p) d -> p c d", p=128),
        )
        qb = loadp.tile([128, NQC * D], H16, tag="qb")
        nc.vector.tensor_copy(qb[:], qn[:])
        qt_alt = loadp.tile([128, NQC * D], H16, tag="qt_alt")
        for j in range(NQC // 2):
            nc.sync.dma_start(
                qt_alt[:, j * 128 : (j + 1) * 128],
                qb[:, j * 128 : (j + 1) * 128],
                transpose=True,
            )
        qt2r = const.tile([128, LQ], H16)
        nblk = NQC // 2
        for parity in range(2):
            src = qt_alt[parity * 64 : (parity + 1) * 64, :].rearrange(
                "p (j c) -> p j c", c=128
            )
            for half in range(2):
                dst = qt2r[half * 64 : (half + 1) * 64, :].rearrange(
                    "p (j c) -> p j c", c=256
                )[:, :, parity * 128 : (parity + 1) * 128]
                nc.sync.dma_start(dst, src)

        if setup_only:
            return

        for ti in range(NQT):
            acc = ps_acc.tile([DP, QTW], FP32)
            ngrp = (NKC + GRP - 1) // GRP
            s_tiles = [None] * ngrp
            filled = [0] * ngrp
            qsl = slice(ti * QTW, (ti + 1) * QTW)

            def consume_group(g):
                gsz = min(GRP, NKC - g * GRP)
                p = pp.tile([128, GRP * QTW], FP32, tag="p")
                nc.scalar.activation(
                    p[:, : gsz * QTW], s_tiles[g][:, : gsz * QTW], Exp, scale=SCALE
                )
                for s in range(gsz):
                    c = g * GRP + s
                    nc.tensor.matmul(
                        acc[:],
                        vp[:, c * DP : (c + 1) * DP],
                        p[:, s * QTW : (s + 1) * QTW],
                        start=(c == 0),
                        stop=(c == NKC - 1),
                    )

            for c in range(NKC):
                g, slot = divmod(c, GRP)
                if s_tiles[g] is None:
                    s_tiles[g] = ps_scores.tile(
                        [128, GRP * QTW], FP32, tag="scores", name="sgrp"
                    )
                half, j = c % 2, c // 2
                nc.tensor.matmul(
                    s_tiles[g][:, slot * QTW : (slot + 1) * QTW],
                    kt2[half * 64 : (half + 1) * 64, j * 128 : (j + 1) * 128],
                    qt2r[half * 64 : (half + 1) * 64, qsl],
                    start=True,
                    stop=True,
                )
                filled[g] += 1
                if filled[g] == min(GRP, NKC - g * GRP):
                    consume_group(g)

            # Epilogue: transpose out^T back to [q, DP], divide by denom.
            accs = outp.tile([DP, QTW], FP32, tag="accs")
            nc.vector.tensor_copy(accs[:], acc[:])
            for j in range(QTW // 128):
                tr = ps_tr.tile([128, DP], FP32)
                nc.tensor.transpose(tr[:], accs[:, j * 128 : (j + 1) * 128],
                                    ident[:DP, :DP])
                rc = outp.tile([128, 1], FP32, tag="rc")
                nc.vector.reciprocal(rc[:], tr[:, D:DP])
                ot = outp.tile([128, D], FP32, tag="ot")
                nc.vector.tensor_scalar_mul(ot[:], tr[:, :D], rc[:])
                r0 = ti * QTW + j * 128
                nc.sync.dma_start(o[r0 : r0 + 128, :], ot[:])


_BUILT = {}

def emit_attention_v3(tc, q, k, v, o, LQ, LK, D, QTW, GRP=3, setup_only=False,
                      loop=None, pipelined=False, stage="full", pbufs=4,
                      sbufs=2, depth=1, xq=False):
    """v3: fp32 everywhere. Packed K^T layout (even chunks on partitions
    0-63, odd on 64-127) built with PE transposes (tile_position=(0,64)
    for the odd chunks); Q^T replicated to both halves with one
    SBUF->SBUF DMA. Row-tiled concurrent scores matmuls + GRP-chunk exp
    groups. `loop` (optional contextmanager factory) wraps the main loop
    for repeat-timing builds; setup stays outside.
    """
    import contextlib

    nc = tc.nc
    assert D == 64 and QTW % 128 == 0
    NKC = LK // 128
    NQC = LQ // 128
    NQT = LQ // QTW
    DP = D + 1
    SCALE = 1.0 / math.sqrt(D)
    Exp = mybir.ActivationFunctionType.Exp
    loop = loop or contextlib.nullcontext

    from contextlib import ExitStack

    with ExitStack() as ctx:
        const = ctx.enter_context(tc.tile_pool(name="const", bufs=1))
        loadp = ctx.enter_context(tc.tile_pool(name="load", bufs=1))
        pp = ctx.enter_context(tc.tile_pool(name="p", bufs=pbufs))
        outp = ctx.enter_context(tc.tile_pool(name="outs", bufs=4))
        ps_scores = ctx.enter_context(
            tc.tile_pool(name="ps_scores", bufs=sbufs, space="PSUM")
        )
        ps_acc = ctx.enter_context(tc.tile_pool(name="ps_acc", bufs=1, space="PSUM"))
        ps_tr = ctx.enter_context(tc.tile_pool(name="ps_tr", bufs=1, space="PSUM"))

        ident = const.tile([128, 128], FP32)
        make_identity(nc, ident[:])

        vp = const.tile([128, NKC * DP], FP32)
        vp3 = vp[:].rearrange("p (c w) -> p c w", w=DP)
        nc.sync.dma_start(vp3[:, :, 0:D], v.rearrange("(c p) d -> p c d", p=128))
        nc.gpsimd.memset(vp3[:, :, D:DP], 1.0)

        kn = loadp.tile([128, NKC * D], FP32, tag="kn")
        nc.sync.dma_start(
            kn[:].rearrange("p (c d) -> p c d", d=D),
            k.rearrange("(c p) d -> p c d", p=128),
        )
        qn = loadp.tile([128, NQC * D], FP32, tag="qn")
        nc.sync.dma_start(
            qn[:].rearrange("p (c d) -> p c d", d=D),
            q.rearrange("(c p) d -> p c d", p=128),
        )

        # Packed K^T: one [128,128] transpose per chunk PAIR. Transposing
        # kn[:, j*128:(j+1)*128] (chunks 2j | 2j+1 side by side) lands
        # chunk 2j's K^T on partitions 0-63 and chunk 2j+1's on 64-127.
        kt2 = const.tile([128, (NKC // 2) * 128], FP32)
        for j in range(NKC // 2):
            tr = ps_scores.tile([128, 128], FP32, tag="scores", name="trs")
            nc.tensor.transpose(tr[:], kn[:, j * 128 : (j + 1) * 128], ident[:])
            nc.vector.tensor_copy(kt2[:, j * 128 : (j + 1) * 128], tr[:])

        # Q^T on partitions 0-63, then replicate to 64-127 with one DMA.
        qt2r = const.tile([128, LQ], FP32)
        for c in range(NQC):
            tr = ps_scores.tile([128, 128], FP32, tag="scores", name="trs")
            nc.tensor.transpose(
                tr[0:64, :], qn[:, c * D : (c + 1) * D], ident[:]
            )
            nc.vector.tensor_copy(
                qt2r[0:64, c * 128 : (c + 1) * 128], tr[0:64, :]
            )
        nc.sync.dma_start(qt2r[64:128, :], qt2r[0:64, :])

        if setup_only:
            return

        if xq:
            # Cross-qtile pipelining: one flat stream of (qtile, group) work
            # consumed with lag `depth`, so PE stays fed across qtile
            # boundaries; each qtile's epilogue is emitted right after its
            # last accumulate, overlapping the next qtile's scores matmuls.
            ngrp = (NKC + GRP - 1) // GRP
            acc_t = [None] * NQT
            s_map = {}

            def epilogue(ti):
                accs = outp.tile([DP, QTW], FP32, tag="accs", name="accs")
                nc.vector.tensor_copy(accs[:], acc_t[ti][:])
                for j in range(QTW // 128):
                    tr = ps_tr.tile([128, DP], FP32, name="tr")
                    nc.tensor.transpose(tr[:], accs[:, j * 128 : (j + 1) * 128],
                                        ident[:DP, :DP])
                    rc = outp.tile([128, 1], FP32, tag="rc", name="rc")
                    nc.vector.reciprocal(rc[:], tr[:, D:DP])
                    ot = outp.tile([128, D], FP32, tag="ot", name="ot")
                    nc.vector.tensor_scalar_mul(ot[:], tr[:, :D], rc[:])
                    r0 = ti * QTW + j * 128
                    nc.sync.dma_start(o[r0 : r0 + 128, :], ot[:])

            def consume(key):
                ti, g = key
                if acc_t[ti] is None:
                    acc_t[ti] = ps_acc.tile([DP, QTW], FP32, name="acc")
                gsz = min(GRP, NKC - g * GRP)
                p = pp.tile([128, GRP * QTW], FP32, tag="p", name="pg")
                nc.scalar.activation(
                    p[:, : gsz * QTW], s_map[key][:, : gsz * QTW], Exp,
                    scale=SCALE,
                )
                for s in range(gsz):
                    c = g * GRP + s
                    nc.tensor.matmul(
                        acc_t[ti][:],
                        vp[:, c * DP : (c + 1) * DP],
                        p[:, s * QTW : (s + 1) * QTW],
                        start=(c == 0),
                        stop=(c == NKC - 1),
                    )
                if g == ngrp - 1:
                    epilogue(ti)

            with loop():
                pend = []
                for ti in range(NQT):
                    qsl = slice(ti * QTW, (ti + 1) * QTW)
                    for g in range(ngrp):
                        gsz = min(GRP, NKC - g * GRP)
                        s_map[(ti, g)] = ps_scores.tile(
                            [128, GRP * QTW], FP32, tag="scores", name="sgrp"
                        )
                        for i in range(gsz):
                            c = g * GRP + i
                            half, j = c % 2, c // 2
                            nc.tensor.matmul(
                                s_map[(ti, g)][:, i * QTW : (i + 1) * QTW],
                                kt2[half * 64 : (half + 1) * 64,
                                    j * 128 : (j + 1) * 128],
                                qt2r[half * 64 : (half + 1) * 64, qsl],
                                start=True,
                                stop=True,
                            )
                        pend.append((ti, g))
                        if len(pend) > depth:
                            consume(pend.pop(0))
                for key in pend:
                    consume(key)
            return

        with loop():
            for ti in range(NQT):
                acc = (
                    ps_acc.tile([DP, QTW], FP32, name="acc")
                    if stage in ("full", "noepi", "fakep")
                    else None
                )
                ngrp = (NKC + GRP - 1) // GRP
                s_tiles = [None] * ngrp
                qsl = slice(ti * QTW, (ti + 1) * QTW)

                def consume_group(g):
                    if stage == "mm1":
                        return
                    gsz = min(GRP, NKC - g * GRP)
                    p = pp.tile([128, GRP * QTW], FP32, tag="p", name="pg")
                    nc.scalar.activation(
                        p[:, : gsz * QTW], s_tiles[g][:, : gsz * QTW], Exp,
                        scale=SCALE,
                    )
                    if stage == "mm1exp":
                        return
                    for s in range(gsz):
                        c = g * GRP + s
                        rhs = (
                            qt2r[:, 0:QTW]
                            if stage == "fakep"
                            else p[:, s * QTW : (s + 1) * QTW]
                        )
                        nc.tensor.matmul(
                            acc[:],
                            vp[:, c * DP : (c + 1) * DP],
                            rhs,
                            start=(c == 0),
                            stop=(c == NKC - 1),
                        )

                pending = []
                for c in range(NKC):
                    g, slot = divmod(c, GRP)
                    if s_tiles[g] is None:
                        s_tiles[g] = ps_scores.tile(
                            [128, GRP * QTW], FP32, tag="scores", name="sgrp"
                        )
                    half, j = c % 2, c // 2
                    nc.tensor.matmul(
                        s_tiles[g][:, slot * QTW : (slot + 1) * QTW],
                        kt2[half * 64 : (half + 1) * 64, j * 128 : (j + 1) * 128],
                        qt2r[half * 64 : (half + 1) * 64, qsl],
                        start=True,
                        stop=True,
                    )
                    filled = c - g * GRP + 1
                    if filled == min(GRP, NKC - g * GRP):
                        if pipelined:
                            # delay exp+accumulate of group g until later
                            # groups' scores matmuls are in PE's stream, so PE
                            # never idles waiting on ScalarE's exp
                            pending.append(g)
                            if len(pending) > depth:
                                consume_group(pending.pop(0))
                        else:
                            consume_group(g)
                for g in pending:
                    consume_group(g)

                if stage != "full":
                    # keep every engine + DMA alive inside the loop body --
                    # an engine with zero loop instructions deadlocks the
                    # For_i back-edge barrier (observed: device wedge).
                    ka = outp.tile([128, 4], FP32, tag="ka", name="ka")
                    nc.gpsimd.memset(ka[:], 0.0)
                    kb_ = outp.tile([128, 4], FP32, tag="kb", name="kb")
                    nc.vector.tensor_copy(kb_[:], ka[:])
                    kc_ = outp.tile([128, 4], FP32, tag="kc", name="kc")
                    nc.scalar.mul(kc_[:], kb_[:], 1.0)
                    nc.sync.dma_start(o[ti * 128 : ti * 128 + 128, 0:4], kc_[:])
                    continue
                accs = outp.tile([DP, QTW], FP32, tag="accs")
                nc.vector.tensor_copy(accs[:], acc[:])
                for j in range(QTW // 128):
                    tr = ps_tr.tile([128, DP], FP32)
                    nc.tensor.transpose(tr[:], accs[:, j * 128 : (j + 1) * 128],
                                        ident[:DP, :DP])
                    rc = outp.tile([128, 1], FP32, tag="rc")
                    nc.vector.reciprocal(rc[:], tr[:, D:DP])
                    ot = outp.tile([128, D], FP32, tag="ot")
                    nc.vector.tensor_scalar_mul(ot[:], tr[:, :D], rc[:])
                    r0 = ti * QTW + j * 128
                    nc.sync.dma_start(o[r0 : r0 + 128, :], ot[:])


def emit_attention_v6(tc, q, k, v, o, LQ, LK, D, QTW, depth=1, grp=(4, 3),
                      loop=None, setup_only=False, zpad=False, stage="full",
                      unroll=1, accbufs=1, pbufs=4, mm2lag=0, pattern=None):
    """v6: fp16 matmuls everywhere (4x PE speedup over fp32).

    - K^T/Q^T built via DMA-xbar transposes (fp16) in the packed layout
      (even k-chunks on partitions 0-63, odd on 64-127); Q^T replicated to
      both halves. PE runs ONLY matmuls (no transpose-mode switches).
    - Scores fp32 in PSUM; groups alternate sizes grp=(4,3) so the two
      score buffers (4+3=7 banks) + acc (1 bank) fill PSUM exactly.
    - exp on ScalarE reads PSUM fp32, writes SBUF fp16 P with the 1/8
      scale folded in. ScalarE is the bottleneck engine (~1 elem/cyc/lane).
    - MM2 accumulates V'^T @ P into acc [65, 512] fp32 (ones column gives
      the softmax denominator).
    - Epilogue: cast acc to fp16, DMA-xbar transpose back to [q, 65],
      reciprocal+scale on DVE, DMA out. No PSUM, no PE.
    - Cross-qtile flat pipelining with lag `depth` keeps PE/ACT busy
      across q-tile boundaries.
    """
    import contextlib

    nc = tc.nc
    assert D == 64 and QTW == 512
    NKC = LK // 128
    NQC = LQ // 128
    NQT = LQ // QTW
    DP = D + 1
    SCALE = 1.0 / math.sqrt(D)
    Exp = mybir.ActivationFunctionType.Exp
    loop = loop or contextlib.nullcontext

    from contextlib import ExitStack

    with ExitStack() as ctx:
        const = ctx.enter_context(tc.tile_pool(name="const", bufs=1))
        loadp = ctx.enter_context(tc.tile_pool(name="load", bufs=1))
        pp = ctx.enter_context(tc.tile_pool(name="p", bufs=pbufs))
        outp = ctx.enter_context(tc.tile_pool(name="outs", bufs=4))
        ps_sA = ctx.enter_context(
            tc.tile_pool(name="ps_sA", bufs=1, space="PSUM")
        )
        ps_sB = ctx.enter_context(
            tc.tile_pool(name="ps_sB", bufs=1, space="PSUM")
        )
        ps_acc = ctx.enter_context(
            tc.tile_pool(name="ps_acc", bufs=accbufs, space="PSUM")
        )

        # --- K first (it gates MM1): load natural chunk-major in two
        # halves (cast overlaps the second half), cast fp16, one whole-
        # tensor xbar transpose into the packed K^T layout ---
        kn = loadp.tile([128, NKC * D], FP32, tag="kn")
        kn3 = kn[:].rearrange("p (c d) -> p c d", d=D)
        k3 = k.rearrange("(c p) d -> p c d", p=128)
        kb = loadp.tile([128, NKC * D], H16, tag="kb")
        kt2 = const.tile([128, (NKC // 2) * 128], H16)
        H = NKC // 2
        HW_ = H * D
        for h in range(2):
            nc.sync.dma_start(kn3[:, h * H : (h + 1) * H, :],
                              k3[:, h * H : (h + 1) * H, :])
            nc.vector.tensor_copy(kb[:, h * HW_ : (h + 1) * HW_],
                                  kn[:, h * HW_ : (h + 1) * HW_])
            nc.sync.dma_start(
                kt2[:, h * HW_ : (h + 1) * HW_].rearrange(
                    "p (j c) -> p j c", c=128),
                kb[:, h * HW_ : (h + 1) * HW_],
                transpose=True,
            )
        if zpad:
            # Zero-padded K^T: chunk c's K^T occupies partitions
            # (c%2)*64..(c%2)*64+64 of kt2z block c, zeros elsewhere, so
            # every MM1 is a full 128-contraction matmul (same tile mode as
            # MM2 -> no PE tile-mode switches; FWL applies to MM1 weights).
            kt2z = const.tile([128, NKC * 128], H16)
            nc.gpsimd.memset(kt2z[:], 0.0)
            for parity in range(2):
                src = kt2[parity * 64 : (parity + 1) * 64, :].rearrange(
                    "p (j c) -> p j c", c=128
                )
                dst = kt2z[parity * 64 : (parity + 1) * 64, :].rearrange(
                    "p (j c) -> p j c", c=256
                )[:, :, parity * 128 : (parity + 1) * 128]
                nc.sync.dma_start(dst, src)

        # --- Q: load on the ACT HWDGE ring (parallel with K on SP), cast,
        # one block-transpose, then rearrange alternating halves into qt2r
        # with Q^T replicated on both partition halves ---
        qn = loadp.tile([128, NQC * D], FP32, tag="qn")
        nc.scalar.dma_start(
            qn[:].rearrange("p (c d) -> p c d", d=D),
            q.rearrange("(c p) d -> p c d", p=128),
        )
        qb = loadp.tile([128, NQC * D], H16, tag="qb")
        nc.vector.tensor_copy(qb[:], qn[:])
        qt_alt = loadp.tile([128, NQC * D], H16, tag="qt_alt")
        nc.sync.dma_start(
            qt_alt[:].rearrange("p (j c) -> p j c", c=128),
            qb[:],
            transpose=True,
        )
        qt2r = const.tile([128, LQ], H16)
        for parity in range(2):
            src = qt_alt[parity * 64 : (parity + 1) * 64, :].rearrange(
                "p (j c) -> p j c", c=128
            )
            for half in range(2):
                dst = qt2r[half * 64 : (half + 1) * 64, :].rearrange(
                    "p (j c) -> p j c", c=256
                )[:, :, parity * 128 : (parity + 1) * 128]
                eng = nc.sync if half == 0 else nc.scalar
                eng.dma_start(dst, src)

        # --- V: contiguous natural load on the ACT ring; V' = [V | 1] fp16
        # built by strided cast-copy halves + ones memset. Demoted in
        # scheduler priority so it doesn't delay the K^T/Q^T critical path
        # (V is first needed only at the first MM2). ---
        vn = loadp.tile([128, NKC * D], FP32, tag="vn")
        vp16 = const.tile([128, NKC * DP], H16)
        # order hack: the V DMA writes vn, so making vn depend on qt2r (one
        # tiny copy) forces the scheduler to start the big V transfer only
        # after the K^T/Q^T critical path has cleared the DMA engines.
        nc.vector.tensor_copy(vn[0:1, 0:1], qt2r[0:1, 0:1])
        with tc.high_priority(offset=-100000):
            nc.scalar.dma_start(
                vn[:].rearrange("p (c d) -> p c d", d=D),
                v.rearrange("(c p) d -> p c d", p=128),
            )
            vp16c = vp16[:].rearrange("p (c w) -> p c w", w=DP)
            vn3 = vn[:].rearrange("p (c d) -> p c d", d=D)
            nc.gpsimd.memset(vp16c[:, :, D:DP], 1.0)
            for h in range(2):
                nc.vector.tensor_copy(vp16c[:, h * H : (h + 1) * H, 0:D],
                                      vn3[:, h * H : (h + 1) * H, :])

        # Persistent epilogue staging: rows DP:128 are junk-guard zeros for
        # the DMA transpose, memset ONCE here (outside any repeat loop).
        accst = const.tile([128, NQT * QTW], H16)
        nc.gpsimd.memset(accst[D:128, :], 0.0)
        ident = const.tile([128, 128], FP32)
        make_identity(nc, ident[:])

        if setup_only:
            return

        # --- group schedule: per q-tile, chunk groups of alternating size.
        # `pattern` (explicit per-q-tile group sizes, alternating A/B pools)
        # overrides the greedy (grp cyclic) split. Even-length patterns keep
        # the A/B ping-pong alive across q-tile boundaries. ---
        if pattern is not None:
            assert sum(pattern) == NKC
            sizes = list(pattern)
        else:
            sizes = []
            c0 = 0
            while c0 < NKC:
                gsz = min(grp[len(sizes) % len(grp)], NKC - c0)
                sizes.append(gsz)
                c0 += gsz
        groups = []
        for it in range(unroll):
            for ti in range(NQT):
                c0 = 0
                for gi, gsz in enumerate(sizes):
                    groups.append((it, ti, gi, c0, gsz))
                    c0 += gsz

        acc_t = {}
        smap = {}

        def emit_mm1(key):
            it, ti, g, c0, gsz = key
            pool = ps_sA if g % 2 == 0 else ps_sB
            s = pool.tile([128, gsz * QTW], FP32, name=f"s{g % 2}")
            smap[key] = s
            qsl = slice(ti * QTW, (ti + 1) * QTW)
            for i in range(gsz):
                c = c0 + i
                half, j = c % 2, c // 2
                if zpad:
                    nc.tensor.matmul(
                        s[:, i * QTW : (i + 1) * QTW],
                        kt2z[:, c * 128 : (c + 1) * 128],
                        qt2r[:, qsl],
                        start=True, stop=True,
                    )
                else:
                    nc.tensor.matmul(
                        s[:, i * QTW : (i + 1) * QTW],
                        kt2[half * 64 : (half + 1) * 64, j * 128 : (j + 1) * 128],
                        qt2r[half * 64 : (half + 1) * 64, qsl],
                        start=True, stop=True,
                    )

        def epilogue(itti):
            it, ti = itti
            eng = nc.sync if ti % 2 == 0 else nc.scalar
            ot = outp.tile([128, (QTW // 128) * D], FP32, tag="ot", name="ot")
            if ti == NQT - 1:
                # Last q-tile: its epilogue tail is exposed (nothing left to
                # overlap with), so use the shorter PE-transpose chain. PE is
                # idle here and the B score bank was just freed by the last
                # exp, so borrow it for the transpose outputs.
                accsf = outp.tile([DP, QTW], FP32, tag="accsf", name="accsf")
                nc.vector.tensor_copy(accsf[:], acc_t[itti][:])
                # reuse the B score buffer (same name/shape -> same slot)
                trt = ps_sB.tile([128, min(grp) * QTW], FP32, name="s1")
                tr4 = trt[:, : (QTW // 128) * DP]
                for j in range(QTW // 128):
                    nc.tensor.transpose(
                        tr4[:, j * DP : (j + 1) * DP],
                        accsf[:, j * 128 : (j + 1) * 128],
                        ident[:DP, :DP],
                    )
                for j in range(QTW // 128):
                    rc = outp.tile([128, 1], FP32, tag="rc", name="rc")
                    nc.vector.reciprocal(
                        rc[:], tr4[:, j * DP + D : (j + 1) * DP])
                    nc.vector.tensor_scalar_mul(
                        ot[:, j * D : (j + 1) * D],
                        tr4[:, j * DP : j * DP + D], rc[:],
                    )
            else:
                accs16 = accst[:, ti * QTW : (ti + 1) * QTW]
                nc.vector.tensor_copy(accs16[0:DP, :], acc_t[itti][:])
                # whole-tile xbar transpose: block j of accs16 -> ot16[:, j, :]
                ot16 = outp.tile([128, QTW], H16, tag="ot16", name="ot16")
                eng.dma_start(
                    ot16[:].rearrange("p (j c) -> p j c", c=128),
                    accs16,
                    transpose=True,
                )
                for j in range(QTW // 128):
                    rc = outp.tile([128, 1], FP32, tag="rc", name="rc")
                    nc.vector.reciprocal(
                        rc[:], ot16[:, j * 128 + D : j * 128 + DP])
                    nc.vector.tensor_scalar_mul(
                        ot[:, j * D : (j + 1) * D],
                        ot16[:, j * 128 : j * 128 + D], rc[:],
                    )
            # one out-DMA for the whole q-tile: o rows = ti*QTW + j*128 + p
            # (SBUF-side AP keeps partitions as dim 0)
            eng.dma_start(
                o[ti * QTW : (ti + 1) * QTW, :].rearrange(
                    "(j p) d -> p j d", p=128),
                ot[:].rearrange("p (j d) -> p j d", d=D),
            )
            # Pool keepalive (memsets were hoisted out of the loop; an
            # engine with no loop instructions deadlocks the For_i barrier)
            ka = outp.tile([128, 1], FP32, tag="ka", name="ka")
            nc.gpsimd.memset(ka[:], 0.0)

        pmap = {}

        def do_exp(key):
            it, ti, g, c0, gsz = key
            s = smap.pop(key)
            if stage == "mm1":
                return
            p = pp.tile([128, gsz * QTW], H16, tag="p", name="pg")
            pmap[key] = p
            nc.scalar.activation(p[:], s[:], Exp, scale=SCALE)

        def do_mm2(key):
            it, ti, g, c0, gsz = key
            if stage in ("mm1", "mm1exp"):
                pmap.pop(key, None)
                return
            if (it, ti) not in acc_t:
                acc_t[(it, ti)] = ps_acc.tile([DP, QTW], FP32, name="acc")
            p = pmap.pop(key)
            for i in range(gsz):
                c = c0 + i
                nc.tensor.matmul(
                    acc_t[(it, ti)][:],
                    vp16[:, c * DP : (c + 1) * DP],
                    p[:, i * QTW : (i + 1) * QTW],
                    start=(c == 0),
                    stop=(c == NKC - 1),
                )
            if c0 + gsz == NKC and stage == "full":
                epilogue((it, ti))

        with loop():
            n = len(groups)
            for i in range(n + depth + mm2lag):
                if i < n:
                    emit_mm1(groups[i])
                j = i - depth
                if 0 <= j < n:
                    do_exp(groups[j])
                j = i - depth - mm2lag
                if 0 <= j < n:
                    do_mm2(groups[j])
            if stage != "full":
                # keep every engine + DMA alive inside the loop body (an
                # engine with zero loop instructions deadlocks the For_i
                # back-edge barrier).
                ka = outp.tile([128, 4], FP32, tag="ka", name="ka")
                nc.gpsimd.memset(ka[:], 0.0)
                kb_ = outp.tile([128, 4], FP32, tag="kb", name="kb")
                nc.vector.tensor_copy(kb_[:], ka[:])
                kc_ = outp.tile([128, 4], FP32, tag="kc", name="kc")
                nc.scalar.mul(kc_[:], kb_[:], 1.0)
                nc.sync.dma_start(o[0:128, 0:4], kc_[:])


import functools

EMITTERS = {
    "v1": emit_attention,
    "v1t": functools.partial(emit_attention, GRP=3),
    "v1set": functools.partial(emit_attention, setup_only=True),
    "v2": emit_attention_v2,
    "v2g2": functools.partial(emit_attention_v2, GRP=2),
    "v2set": functools.partial(emit_attention_v2, setup_only=True),
    "v3": emit_attention_v3,
    "v3p": functools.partial(emit_attention_v3, pipelined=True),
    "v3g2": functools.partial(emit_attention_v3, GRP=2),
    "v3set": functools.partial(emit_attention_v3, setup_only=True),
    "v3mm1": functools.partial(emit_attention_v3, stage="mm1"),
    "v3mm1e": functools.partial(emit_attention_v3, stage="mm1exp"),
    "v3mm1ep": functools.partial(emit_attention_v3, stage="mm1exp",
                                 pipelined=True),
    "v3noepi": functools.partial(emit_attention_v3, stage="noepi",
                                 pipelined=True),
    "v3fakep": functools.partial(emit_attention_v3, stage="fakep",
                                 pipelined=True),
    "v3pb": functools.partial(emit_attention_v3, pipelined=True, pbufs=8),
    "v4": functools.partial(emit_attention_v3, GRP=2, sbufs=3, depth=2,
                            pbufs=6, pipelined=True),
    "v4d1": functools.partial(emit_attention_v3, GRP=2, sbufs=3, depth=1,
                              pbufs=6, pipelined=True),
    "v5": functools.partial(emit_attention_v3, GRP=2, sbufs=3, depth=1,
                            pbufs=6, xq=True),
    "v6": emit_attention_v6,
    "v6g3": functools.partial(emit_attention_v6, grp=(3,)),
    "v6g3a2": functools.partial(emit_attention_v6, grp=(3,), accbufs=2),
    "v6d2": functools.partial(emit_attention_v6, depth=2),
    "v6z": functools.partial(emit_attention_v6, zpad=True),
    "v6zd2": functools.partial(emit_attention_v6, zpad=True, depth=2),
    "v6mm1": functools.partial(emit_attention_v6, stage="mm1"),
    "v6mm1e": functools.partial(emit_attention_v6, stage="mm1exp"),
    "v6noepi": functools.partial(emit_attention_v6, stage="noepi"),
    "v6zmm1": functools.partial(emit_attention_v6, zpad=True, stage="mm1"),
    "v6zmm1e": functools.partial(emit_attention_v6, zpad=True, stage="mm1exp"),
}
PAT10 = (4, 3, 4, 3, 4, 3, 4, 3, 2, 2)
EMITTERS.update({
    # mm2 lagged one group behind exp so the PE FIFO runs the next group's
    # MM1 batch before MM2(g) (whose exp gates it) - removes ACT cadence gaps
    "v7": functools.partial(emit_attention_v6, mm2lag=1),
    # + even group count per q-tile keeps A/B ping-pong across boundaries
    "v7e": functools.partial(emit_attention_v6, mm2lag=1, pattern=PAT10),
    "v7e2": functools.partial(emit_attention_v6, mm2lag=2, pattern=PAT10,
                              pbufs=6),
    # pair-interleaved q-tiles, (3,3) groups, 2 acc banks
    "v7i": functools.partial(emit_attention_v6, mm2lag=1, grp=(3,),
                             accbufs=2),
    "v7noepi": functools.partial(emit_attention_v6, mm2lag=1, pattern=PAT10,
                                 stage="noepi"),
    "v7mm1e": functools.partial(emit_attention_v6, mm2lag=1, pattern=PAT10,
                                stage="mm1exp"),
})
for _n in list(EMITTERS):
    _e = EMITTERS[_n]
    if not (isinstance(_e, functools.partial)
            and _e.func is emit_attention_v6) and _e is not emit_attention_v6:
        continue
    for _u in (2, 3, 5):
        EMITTERS[f"{_n}u{_u}"] = functools.partial(
            _e.func if isinstance(_e, functools.partial) else _e,
            unroll=_u,
            **(_e.keywords if isinstance(_e, functools.partial) else {}),
        )
LOOP_SPLIT = {"v3", "v3p", "v3g2", "v3mm1", "v3mm1e", "v3mm1ep",
              "v3noepi", "v3fakep", "v3pb", "v4", "v4d1", "v5"} | {
    n for n, e in EMITTERS.items()
    if isinstance(e, functools.partial) and e.func is emit_attention_v6
} | {"v6"}
DEFAULT_VARIANT = "v7e"


def _build(repeat=1, variant=None):
    """Build the per-core module. repeat>1 wraps the body in a hardware
    For_i loop (used only for on-device timing; grading uses repeat=1)."""
    variant = variant or DEFAULT_VARIANT
    key = (repeat, variant)
    if key not in _BUILT:
        emitter = EMITTERS[variant]
        nc = bacc.Bacc("TRN2", target_bir_lowering=False, debug=False)
        q = nc.dram_tensor("q", [LQ, D], FP32, kind="ExternalInput")
        k = nc.dram_tensor("k", [LK, D], FP32, kind="ExternalInput")
        v = nc.dram_tensor("v", [LK, D], FP32, kind="ExternalInput")
        o = nc.dram_tensor("o", [LQ, D], FP32, kind="ExternalOutput")
        engines = (
            mybir.EngineType.PE,
            mybir.EngineType.Activation,
            mybir.EngineType.DVE,
            mybir.EngineType.SP,
            mybir.EngineType.Pool,
        )
        with tile.TileContext(nc) as tc:
            if repeat == 1:
                emitter(tc, q[:], k[:], v[:], o[:], LQ, LK, D, QTW=512)
            elif variant in LOOP_SPLIT:
                emitter(
                    tc, q[:], k[:], v[:], o[:], LQ, LK, D, QTW=512,
                    loop=lambda: tc.For_i(0, repeat, 1, hint_engines=engines),
                )
            else:
                with tc.For_i(0, repeat, 1, hint_engines=engines):
                    emitter(tc, q[:], k[:], v[:], o[:], LQ, LK, D, QTW=512)
        nc.compile()
        _BUILT[key] = nc
    return _BUILT[key]


def _shard_inputs(query, keys, values):
    in_maps = []
    for c in range(N_CORES):
        b, h = c // 2, c % 2
        in_maps.append({
            "q": np.ascontiguousarray(query[b, h * LQ : (h + 1) * LQ, :],
                                      dtype=np.float32),
            "k": np.ascontiguousarray(keys[b], dtype=np.float32),
            "v": np.ascontiguousarray(values[b], dtype=np.float32),
        })
    return in_maps


def run_sharded(query, keys, values, trace=False, repeat=1, variant=None):
    """Run on 8 cores; returns (full_output, BassKernelResults)."""
    from concourse.bass_utils import run_bass_kernel_spmd

    nc = _build(repeat, variant)
    in_maps = _shard_inputs(query, keys, values)
    res = run_bass_kernel_spmd(nc, in_maps, list(range(N_CORES)), trace=trace)
    out = np.empty((B, LQ_FULL, D), np.float32)
    for c in range(N_CORES):
        b, h = c // 2, c % 2
        out[b, h * LQ : (h + 1) * LQ, :] = res.results[c]["o"]
    return out, res


def kernel(query, keys, values):
    out, _ = run_sharded(np.asarray(query), np.asarray(keys), np.asarray(values))
    return out

